# revision 32
# baseline (speedup 1.0000x reference)
"""Multi-head attention (bs=4, seq=2048, hidden=1024, 16 heads) on 8 trn2 cores.

Sharding: core = (batch b, head-group g) with 4 batches x 2 groups of 8 heads.
Each core computes QKV projections for its head slice, causal+padded softmax
attention, and a partial output projection; the host sums the two partial
outputs per batch and adds o_b.

v2 layout notes:
  - bf16 weights/activations in SBUF (fp32 accumulate in PSUM); host converts.
  - phase-1 projections run in four 512-seq quarters, emitted interleaved
    with the four 512-query attention windows so the Tile scheduler can keep
    the PE busy with projection matmuls while ScalarE runs softmax exps.
  - score matmuls for a head pair issue to disjoint 64-row PE groups
    (base partitions 0 and 64) so they execute concurrently.
  - exp runs once per (chunk, head-pair) as a single wide ACT op over a
    [128, 2, w] access pattern; padding mask rides as a per-partition bias.
  - softmax division: denominator rows leave PSUM via ScalarE ln, the
    reciprocal is exp(-ln) (same ACT table set), GpSimd broadcasts it
    across partitions, DVE does the final multiply.
"""
import os
import sys

for _p in ("/opt/trn_rl_repo",):
    if _p not in sys.path:
        sys.path.insert(0, _p)

import numpy as np

HID = 1024
HEADS = 16
D = 64
BS = 4
SEQ = 2048
NCORES = 8
HG = 2             # head groups (tensor-parallel axis)
HPG = HEADS // HG  # 8 heads per core
NPAIR = HPG // 2   # 4 head pairs per core
OG = HPG * D       # 512 projection dims per core
KC = HID // 128    # 8 hidden chunks
W = 512            # query window
NW = SEQ // W      # 4 windows (== phase-1 quarters)
SC = SEQ // 128    # 16 key chunks
SCALE = 1.0 / np.sqrt(D)

_compiled = None


def _build():
    import concourse.tile as tile
    from concourse import bacc, mybir

    F32 = mybir.dt.float32
    BF16 = mybir.dt.bfloat16
    AF = mybir.ActivationFunctionType
    Alu = mybir.AluOpType

    nc = bacc.Bacc("TRN2", target_bir_lowering=False, debug=False,
                   num_devices=NCORES)

    xT_d = nc.dram_tensor("xT", [HID, SEQ], BF16, kind="ExternalInput").ap()
    wqT_d = nc.dram_tensor("wqT", [HID, OG], BF16, kind="ExternalInput").ap()
    wkT_d = nc.dram_tensor("wkT", [HID, OG], BF16, kind="ExternalInput").ap()
    wvT_d = nc.dram_tensor("wvT", [HID, OG], BF16, kind="ExternalInput").ap()
    woT_d = nc.dram_tensor("woT", [OG, HID], BF16, kind="ExternalInput").ap()
    qb_d = nc.dram_tensor("qb", [128, 4], F32, kind="ExternalInput").ap()
    kb_d = nc.dram_tensor("kb", [128, 4], F32, kind="ExternalInput").ap()
    vb_d = nc.dram_tensor("vb", [1, OG], BF16, kind="ExternalInput").ap()
    kmask_d = nc.dram_tensor("kmask", [128, SC], F32, kind="ExternalInput").ap()
    out_d = nc.dram_tensor("out", [SEQ, HID], BF16,
                           kind="ExternalOutput").ap()

    with tile.TileContext(nc) as tc:
        with tc.tile_pool(name="const", bufs=1) as cp, \
             tc.tile_pool(name="wq", bufs=1) as wqp, \
             tc.tile_pool(name="wk", bufs=1) as wkp, \
             tc.tile_pool(name="wv", bufs=1) as wvp, \
             tc.tile_pool(name="wo", bufs=1) as wop, \
             tc.tile_pool(name="qT", bufs=1) as qTp, \
             tc.tile_pool(name="kT", bufs=1) as kTp, \
             tc.tile_pool(name="v", bufs=1) as vp, \
             tc.tile_pool(name="attnT", bufs=1) as aTp, \
             tc.tile_pool(name="x", bufs=2) as xp, \
             tc.tile_pool(name="ph2", bufs=1) as p2, \
             tc.tile_pool(name="ph3", bufs=1) as p3, \
             tc.tile_pool(name="psB", bufs=3, space="PSUM") as psB, \
             tc.tile_pool(name="psC", bufs=1, space="PSUM") as psC:

            # ---------------- constants ----------------
            ones_f = cp.tile([128, 128], F32, tag="ones_f")
            nc.gpsimd.memset(ones_f[:, :], 1.0)
            onesb = cp.tile([128, 128], BF16, tag="onesb")
            nc.scalar.copy(onesb[:, :], ones_f[:, :])
            # tri01[p, j] = 1 if j >= p else 0 (keep keys <= query), two
            # adjacent copies so one 3D-AP multiply masks both heads.
            tri01_f = cp.tile([128, 128], F32, tag="tri01_f")
            nc.gpsimd.affine_select(tri01_f[:, :], ones_f[:, :],
                                    pattern=[[1, 128]],
                                    compare_op=Alu.is_ge, fill=0.0,
                                    base=0, channel_multiplier=-1)
            tri2 = cp.tile([128, 256], BF16, tag="tri2")
            nc.scalar.copy(tri2[:, 0:128], tri01_f[:, :])
            nc.scalar.copy(tri2[:, 128:256], tri01_f[:, :])
            qb_s = cp.tile([128, 4], F32, tag="qb")
            nc.sync.dma_start(qb_s[:, :], qb_d[:, :])
            kb_s = cp.tile([128, 4], F32, tag="kb")
            nc.sync.dma_start(kb_s[:, :], kb_d[:, :])
            vb_s = cp.tile([1, OG], BF16, tag="vb")
            nc.sync.dma_start(vb_s[:, :], vb_d[:, :])
            kmask_s = cp.tile([128, SC], F32, tag="km")
            nc.sync.dma_start(kmask_s[:, :], kmask_d[:, :])

            def load_x_quarter(q):
                qs = q * W
                xT_t = []
                for kc in range(KC):
                    xt = xp.tile([128, W], BF16, tag=f"xT{kc}",
                                 name=f"xT{q}_{kc}")
                    nc.sync.dma_start(
                        xt[:, :], xT_d[kc * 128:(kc + 1) * 128, qs:qs + W])
                    xT_t.append(xt)
                return xT_t

            # quarter-0 activations first so the first projection matmuls
            # can start while the bulk of the weights still stream in
            x0_t = load_x_quarter(0)

            # ---------------- weights (loaded once) ----------------
            # q/k first (quarter-0 needs them before v), v next, wo last
            wq_t, wk_t, wv_t = [], [], []
            for kc in range(KC):
                for lst, pool, src in ((wq_t, wqp, wqT_d),
                                       (wk_t, wkp, wkT_d)):
                    wt = pool.tile([128, OG], BF16, tag=f"w{kc}",
                                   name=f"w{len(lst)}_{kc}")
                    nc.sync.dma_start(wt[:, :],
                                      src[kc * 128:(kc + 1) * 128, :])
                    lst.append(wt)
            for kc in range(KC):
                wt = wvp.tile([128, OG], BF16, tag=f"w{kc}", name=f"wv_{kc}")
                nc.sync.dma_start(wt[:, :],
                                  wvT_d[kc * 128:(kc + 1) * 128, :])
                wv_t.append(wt)
            wo_t = []
            for kc in range(4):
                wo = wop.tile([128, HID], BF16, tag=f"wo{kc}")
                nc.sync.dma_start(wo[:, :], woT_d[kc * 128:(kc + 1) * 128, :])
                wo_t.append(wo)

            # ---------------- persistent activation tiles ----------------
            # split per quarter/window so dependency tracking matches the
            # producer/consumer granularity (window w needs only quarter w
            # of q, quarters <= w of k/v, and writes only window w of attnT)
            qT_t = [[qTp.tile([128, W], BF16, tag=f"qT{i}_{q}",
                              name=f"qT{i}_{q}") for q in range(NW)]
                    for i in range(NPAIR)]
            kT_t = [[kTp.tile([128, W], BF16, tag=f"kT{i}_{q}",
                              name=f"kT{i}_{q}") for q in range(NW)]
                    for i in range(NPAIR)]
            v_t = [vp.tile([128, HPG * 65], BF16, tag=f"v{i}", name=f"v{i}")
                   for i in range(SC)]
            for i in range(SC):
                vv = v_t[i].rearrange("p (h c) -> p h c", c=65)
                nc.gpsimd.memset(vv[:, :, 64:65], 1.0)
            attnT_t = [[aTp.tile([128, W], BF16, tag=f"aT{i}_{q}",
                                 name=f"aT{i}_{q}") for q in range(NW)]
                       for i in range(NPAIR)]

            def phase1_quarter(q, xT_t=None):
                qs = q * W
                if xT_t is None:
                    xT_t = load_x_quarter(q)
                # Q/K projections: out partitions = proj dims, cols = seq
                for w_t, o_t, bias in ((wq_t, qT_t, qb_s), (wk_t, kT_t, kb_s)):
                    for oc in range(4):
                        pqk = psC.tile([128, W], F32, tag="c512", name="pqk")
                        for kc in range(KC):
                            nc.tensor.matmul(
                                pqk[:, :],
                                w_t[kc][:, oc * 128:(oc + 1) * 128],
                                xT_t[kc][:, :],
                                start=(kc == 0), stop=(kc == KC - 1))
                        nc.vector.tensor_scalar_add(
                            o_t[oc][q][:, :], pqk[:, :],
                            bias[:, oc:oc + 1])
                # V projection: out partitions = seq chunk, cols = proj dims
                for sc in range(4):
                    scg = 4 * q + sc
                    pv = psC.tile([128, OG], F32, tag="c512", name="pv")
                    for kc in range(KC):
                        nc.tensor.matmul(
                            pv[:, :],
                            xT_t[kc][:, sc * 128:(sc + 1) * 128],
                            wv_t[kc][:, :],
                            start=(kc == 0), stop=False)
                    nc.tensor.matmul(pv[:, :], onesb[0:1, :], vb_s[0:1, :],
                                     start=False, stop=True)
                    src = pv.rearrange("p (h c) -> p h c", c=64)
                    dst = v_t[scg].rearrange("p (h c) -> p h c", c=65)
                    nc.vector.tensor_copy(dst[:, :, 0:64], src[:, :, :])

            def phase2_window(w, psA):
                ws = w * W
                chunks = [(c, 0) for c in range(4 * w)]
                chunks += [(4 * w + i, 128 * i) for i in range(4)]
                last = len(chunks) - 1
                for pr in range(NPAIR):
                    he = 2 * pr       # even head (rows 0:64)
                    at_e = psB.tile([128, W], F32, tag="b512", name="at_e")
                    at_o = psB.tile([128, W], F32, tag="b512", name="at_o")
                    for idx, (c, off) in enumerate(chunks):
                        n = W - off
                        kTq = kT_t[pr][c // 4]
                        kc0 = (c % 4) * 128
                        sp = psA.tile([128, 2 * W], F32, tag="sp", name="sp")
                        sp3 = sp.rearrange("p (g c) -> p g c", g=2)
                        nc.tensor.matmul(
                            sp[:, off:W],
                            kTq[0:64, kc0:kc0 + 128],
                            qT_t[pr][w][0:64, off:W],
                            start=True, stop=True)
                        nc.tensor.matmul(
                            sp[:, W + off:2 * W],
                            kTq[64:128, kc0:kc0 + 128],
                            qT_t[pr][w][64:128, off:W],
                            start=True, stop=True)
                        et = p2.tile([128, 2 * W], BF16, tag="E", bufs=6)
                        et3 = et.rearrange("p (g c) -> p g c", g=2)
                        nc.scalar.activation(et3[:, :, off:W],
                                             sp3[:, :, off:W], AF.Exp,
                                             bias=kmask_s[:, c:c + 1],
                                             scale=SCALE)
                        if off or c == 4 * w:  # diagonal chunk
                            nc.vector.tensor_mul(
                                et3[:, :, off:off + 128],
                                et3[:, :, off:off + 128],
                                tri2.rearrange("p (g c) -> p g c", g=2))
                        nc.tensor.matmul(
                            at_e[0:65, off:W],
                            v_t[c][:, he * 65:(he + 1) * 65],
                            et[:, off:W],
                            start=(idx == 0), stop=(idx == last))
                        nc.tensor.matmul(
                            at_o[0:65, off:W],
                            v_t[c][:, (he + 1) * 65:(he + 2) * 65],
                            et[:, W + off:2 * W],
                            start=(idx == 0), stop=(idx == last))
                    # softmax division, off the PE critical path:
                    # evict unnormalized att + denominator row quickly
                    # (frees the PSUM slot), reciprocal on idle GpSimd,
                    # broadcast across partitions via a K=1 matmul.
                    for h, at in ((he, at_e), (he + 1, at_o)):
                        attnU = p2.tile([64, W], BF16, tag="aU", bufs=3)
                        nc.vector.tensor_copy(attnU[:, :], at[0:64, :])
                        dnr = p2.tile([128, W], F32, tag="dnr", bufs=2)
                        nc.vector.tensor_copy(dnr[64:65, :], at[64:65, :])
                        # reciprocal: DVE divide costs 8 cyc per FREE elem,
                        # so reshape the row to [128, 4] via DMA first
                        dnT = p2.tile([128, 4], F32, tag="dnT", bufs=2)
                        nc.sync.dma_start(dnT[:, :], dnr[64:65, :])
                        dnTr = p2.tile([128, 4], BF16, tag="dnTr", bufs=2)
                        with nc.allow_low_precision("recip"):
                            nc.vector.reciprocal(dnTr[:, :], dnT[:, :])
                        rcp = p2.tile([128, W], BF16, tag="rcp", bufs=2)
                        nc.sync.dma_start(rcp[64:65, :], dnTr[:, :])
                        # broadcast the reciprocal row back into the (now
                        # dead) at tile -- reuses its PSUM bank, WAW-ordered
                        # behind the two evictions above
                        nc.tensor.matmul(at[0:64, :], onesb[64:65, 0:64],
                                         rcp[64:65, :], start=True, stop=True)
                        if h % 2 == 0:
                            nc.vector.tensor_mul(
                                attnT_t[pr][w][0:64, :],
                                attnU[:, :], at[0:64, :])
                        else:
                            tmp = p2.tile([64, W], BF16, tag="tm", bufs=2)
                            nc.vector.tensor_mul(tmp[:, :], attnU[:, :],
                                                 at[0:64, :])
                            nc.sync.dma_start(attnT_t[pr][w][64:128, :],
                                              tmp[:, :])

            def phase3_window(w, pool, tag):
                # output projection for the sq chunks of window w
                for sc in range(4 * w, 4 * w + 4):
                    sc0 = (sc % 4) * 128
                    ot = p3.tile([128, HID], BF16, tag="ou", bufs=3)
                    for n in range(2):
                        po = pool.tile([128, W], F32, tag=tag, name="po")
                        for kc in range(4):
                            nc.tensor.matmul(
                                po[:, :],
                                attnT_t[kc][w][:, sc0:sc0 + 128],
                                wo_t[kc][:, n * W:(n + 1) * W],
                                start=(kc == 0), stop=(kc == 3))
                        nc.vector.tensor_copy(ot[:, n * W:(n + 1) * W],
                                              po[:, :])
                    nc.sync.dma_start(out_d[sc * 128:(sc + 1) * 128, :],
                                      ot[:, :])

            # interleave projection quarters, attention windows, and output
            # projection so the scheduler can fill PE idle time during
            # ScalarE-bound (softmax) stretches; the last window's output
            # projection runs after the scores pool closes, in a wider pool
            with tc.tile_pool(name="psA", bufs=2, space="PSUM") as psA:
                phase1_quarter(0, x0_t)
                phase2_window(0, psA)
                phase1_quarter(1)
                phase2_window(1, psA)
                phase1_quarter(2)
                phase2_window(2, psA)
                phase3_window(0, psC, "c512")
                phase1_quarter(3)
                phase2_window(3, psA)
                phase3_window(1, psC, "c512")
                phase3_window(2, psC, "c512")
            with tc.tile_pool(name="psD", bufs=4, space="PSUM") as psD:
                phase3_window(3, psD, "d512")

    nc.compile()
    return nc


def kernel(hidden_states, causal_mask, padding_mask,
           q_w, q_b, k_w, k_b, v_w, v_b, o_w, o_b):
    global _compiled
    from concourse.bass_utils import run_bass_kernel_spmd
    import ml_dtypes

    BF = ml_dtypes.bfloat16

    hidden_states = np.asarray(hidden_states, dtype=np.float32)
    padding_mask = np.asarray(padding_mask)
    q_w = np.asarray(q_w, dtype=np.float32)
    k_w = np.asarray(k_w, dtype=np.float32)
    v_w = np.asarray(v_w, dtype=np.float32)
    o_w = np.asarray(o_w, dtype=np.float32)
    q_b = np.asarray(q_b, dtype=np.float32)
    k_b = np.asarray(k_b, dtype=np.float32)
    v_b = np.asarray(v_b, dtype=np.float32)
    o_b = np.asarray(o_b, dtype=np.float32)

    if _compiled is None:
        _compiled = _build()
    nc = _compiled

    in_maps = []
    for b in range(BS):
        xT = np.ascontiguousarray(hidden_states[b].T).astype(BF)
        kmask = np.where(padding_mask[b], np.float32(-30000.0),
                         np.float32(0.0)).astype(np.float32)
        kmask2 = np.ascontiguousarray(kmask.reshape(SC, 128).T)
        for g in range(HG):
            r = slice(g * OG, (g + 1) * OG)
            in_maps.append({
                "xT": xT,
                "wqT": np.ascontiguousarray(q_w[r].T).astype(BF),
                "wkT": np.ascontiguousarray(k_w[r].T).astype(BF),
                "wvT": np.ascontiguousarray(v_w[r].T).astype(BF),
                "woT": np.ascontiguousarray(o_w[:, r].T).astype(BF),
                "qb": np.ascontiguousarray(q_b[r].reshape(4, 128).T),
                "kb": np.ascontiguousarray(k_b[r].reshape(4, 128).T),
                "vb": np.ascontiguousarray(v_b[r].reshape(1, OG)).astype(BF),
                "kmask": kmask2,
            })

    trace = os.environ.get("KERNEL_TRACE") == "1"
    res = run_bass_kernel_spmd(nc, in_maps, core_ids=list(range(NCORES)),
                               trace=trace)
    if trace and res.exec_time_ns is not None:
        print(f"HW exec time: {res.exec_time_ns} ns")
        if res.instructions_and_trace:
            print(f"trace: {res.instructions_and_trace[1]}")

    out = np.empty((BS, SEQ, HID), dtype=np.float32)
    for b in range(BS):
        out[b] = (res.results[2 * b]["out"].astype(np.float32)
                  + res.results[2 * b + 1]["out"].astype(np.float32)
                  + o_b[None, :])
    return out


# revision 37
# speedup vs baseline: 1.1891x; 1.1891x over previous
"""Multi-head attention (bs=4, seq=2048, hidden=1024, 16 heads) on 8 trn2 cores.

Sharding: core = (batch b, head-group g) with 4 batches x 2 groups of 8 heads.
Each core computes QKV projections for its head slice, causal+padded softmax
attention, and a partial output projection; the host sums the two partial
outputs per batch and adds o_b.

v2 layout notes:
  - bf16 weights/activations in SBUF (fp32 accumulate in PSUM); host converts.
  - phase-1 projections run in four 512-seq quarters, emitted interleaved
    with the four 512-query attention windows so the Tile scheduler can keep
    the PE busy with projection matmuls while ScalarE runs softmax exps.
  - score matmuls for a head pair issue to disjoint 64-row PE groups
    (base partitions 0 and 64) so they execute concurrently.
  - exp runs once per (chunk, head-pair) as a single wide ACT op over a
    [128, 2, w] access pattern; padding mask rides as a per-partition bias.
  - softmax division: denominator rows leave PSUM via ScalarE ln, the
    reciprocal is exp(-ln) (same ACT table set), GpSimd broadcasts it
    across partitions, DVE does the final multiply.
"""
import os
import sys

for _p in ("/opt/trn_rl_repo",):
    if _p not in sys.path:
        sys.path.insert(0, _p)

import numpy as np

HID = 1024
HEADS = 16
D = 64
BS = 4
SEQ = 2048
NCORES = 8
HG = 2             # head groups (tensor-parallel axis)
HPG = HEADS // HG  # 8 heads per core
NPAIR = HPG // 2   # 4 head pairs per core
OG = HPG * D       # 512 projection dims per core
KC = HID // 128    # 8 hidden chunks
W = 512            # query window
NW = SEQ // W      # 4 windows (== phase-1 quarters)
SC = SEQ // 128    # 16 key chunks
SCALE = 1.0 / np.sqrt(D)

_compiled = None


def _build():
    import concourse.tile as tile
    from concourse import bacc, mybir

    F32 = mybir.dt.float32
    BF16 = mybir.dt.bfloat16
    AF = mybir.ActivationFunctionType
    Alu = mybir.AluOpType

    nc = bacc.Bacc("TRN2", target_bir_lowering=False, debug=False,
                   num_devices=NCORES)

    xT_d = nc.dram_tensor("xT", [HID, SEQ], BF16, kind="ExternalInput").ap()
    wqT_d = nc.dram_tensor("wqT", [HID, OG], BF16, kind="ExternalInput").ap()
    wkT_d = nc.dram_tensor("wkT", [HID, OG], BF16, kind="ExternalInput").ap()
    wvT_d = nc.dram_tensor("wvT", [HID, OG], BF16, kind="ExternalInput").ap()
    woT_d = nc.dram_tensor("woT", [OG, HID], BF16, kind="ExternalInput").ap()
    qb_d = nc.dram_tensor("qb", [128, 4], F32, kind="ExternalInput").ap()
    kb_d = nc.dram_tensor("kb", [128, 4], F32, kind="ExternalInput").ap()
    vb_d = nc.dram_tensor("vb", [1, OG], BF16, kind="ExternalInput").ap()
    kmask_d = nc.dram_tensor("kmask", [128, SC], F32, kind="ExternalInput").ap()
    out_d = nc.dram_tensor("out", [SEQ, HID], BF16,
                           kind="ExternalOutput").ap()

    with tile.TileContext(nc) as tc:
        with tc.tile_pool(name="const", bufs=1) as cp, \
             tc.tile_pool(name="wq", bufs=1) as wqp, \
             tc.tile_pool(name="wk", bufs=1) as wkp, \
             tc.tile_pool(name="wv", bufs=1) as wvp, \
             tc.tile_pool(name="wo", bufs=1) as wop, \
             tc.tile_pool(name="qT", bufs=1) as qTp, \
             tc.tile_pool(name="kT", bufs=1) as kTp, \
             tc.tile_pool(name="v", bufs=1) as vp, \
             tc.tile_pool(name="attnT", bufs=1) as aTp, \
             tc.tile_pool(name="x", bufs=2) as xp, \
             tc.tile_pool(name="ph2", bufs=1) as p2, \
             tc.tile_pool(name="ph3", bufs=1) as p3, \
             tc.tile_pool(name="psB", bufs=3, space="PSUM") as psB, \
             tc.tile_pool(name="psC", bufs=1, space="PSUM") as psC:

            # ---------------- constants ----------------
            ones_f = cp.tile([128, 128], F32, tag="ones_f")
            nc.gpsimd.memset(ones_f[:, :], 1.0)
            onesb = cp.tile([128, 128], BF16, tag="onesb")
            nc.scalar.copy(onesb[:, :], ones_f[:, :])
            # tri01[p, j] = 1 if j >= p else 0 (keep keys <= query), two
            # adjacent copies so one 3D-AP multiply masks both heads.
            tri01_f = cp.tile([128, 128], F32, tag="tri01_f")
            nc.gpsimd.affine_select(tri01_f[:, :], ones_f[:, :],
                                    pattern=[[1, 128]],
                                    compare_op=Alu.is_ge, fill=0.0,
                                    base=0, channel_multiplier=-1)
            tri2 = cp.tile([128, 256], BF16, tag="tri2")
            nc.scalar.copy(tri2[:, 0:128], tri01_f[:, :])
            nc.scalar.copy(tri2[:, 128:256], tri01_f[:, :])
            qb_s = cp.tile([128, 4], F32, tag="qb")
            nc.sync.dma_start(qb_s[:, :], qb_d[:, :])
            kb_s = cp.tile([128, 4], F32, tag="kb")
            nc.sync.dma_start(kb_s[:, :], kb_d[:, :])
            vb_s = cp.tile([1, OG], BF16, tag="vb")
            nc.sync.dma_start(vb_s[:, :], vb_d[:, :])
            kmask_s = cp.tile([128, SC], F32, tag="km")
            nc.sync.dma_start(kmask_s[:, :], kmask_d[:, :])

            def load_x_quarter(q):
                qs = q * W
                xT_t = []
                for kc in range(KC):
                    xt = xp.tile([128, W], BF16, tag=f"xT{kc}",
                                 name=f"xT{q}_{kc}")
                    nc.sync.dma_start(
                        xt[:, :], xT_d[kc * 128:(kc + 1) * 128, qs:qs + W])
                    xT_t.append(xt)
                return xT_t

            # quarter-0 activations first so the first projection matmuls
            # can start while the bulk of the weights still stream in
            x0_t = load_x_quarter(0)

            # ---------------- weights (loaded once) ----------------
            # q/k first (quarter-0 needs them before v), v next, wo last
            wq_t, wk_t, wv_t = [], [], []
            for kc in range(KC):
                for lst, pool, src in ((wq_t, wqp, wqT_d),
                                       (wk_t, wkp, wkT_d)):
                    wt = pool.tile([128, OG], BF16, tag=f"w{kc}",
                                   name=f"w{len(lst)}_{kc}")
                    nc.sync.dma_start(wt[:, :],
                                      src[kc * 128:(kc + 1) * 128, :])
                    lst.append(wt)
            for kc in range(KC):
                wt = wvp.tile([128, OG], BF16, tag=f"w{kc}", name=f"wv_{kc}")
                nc.sync.dma_start(wt[:, :],
                                  wvT_d[kc * 128:(kc + 1) * 128, :])
                wv_t.append(wt)
            wo_t = []
            for kc in range(4):
                wo = wop.tile([128, HID], BF16, tag=f"wo{kc}")
                nc.sync.dma_start(wo[:, :], woT_d[kc * 128:(kc + 1) * 128, :])
                wo_t.append(wo)

            # ---------------- persistent activation tiles ----------------
            qT_t = [qTp.tile([128, SEQ], BF16, tag=f"qT{i}", name=f"qT{i}")
                    for i in range(NPAIR)]
            kT_t = [kTp.tile([128, SEQ], BF16, tag=f"kT{i}", name=f"kT{i}")
                    for i in range(NPAIR)]
            v_t = [vp.tile([128, HPG * 65], BF16, tag=f"v{i}", name=f"v{i}")
                   for i in range(SC)]
            for i in range(SC):
                vv = v_t[i].rearrange("p (h c) -> p h c", c=65)
                nc.gpsimd.memset(vv[:, :, 64:65], 1.0)
            attnT_t = [aTp.tile([128, SEQ], BF16, tag=f"aT{i}", name=f"aT{i}")
                       for i in range(NPAIR)]

            def phase1_quarter(q, xT_t=None):
                qs = q * W
                if xT_t is None:
                    xT_t = load_x_quarter(q)
                # Q/K projections: out partitions = proj dims, cols = seq
                for w_t, o_t, bias in ((wq_t, qT_t, qb_s), (wk_t, kT_t, kb_s)):
                    for oc in range(4):
                        pqk = psC.tile([128, W], F32, tag="c512", name="pqk")
                        for kc in range(KC):
                            nc.tensor.matmul(
                                pqk[:, :],
                                w_t[kc][:, oc * 128:(oc + 1) * 128],
                                xT_t[kc][:, :],
                                start=(kc == 0), stop=(kc == KC - 1))
                        nc.vector.tensor_scalar_add(
                            o_t[oc][:, qs:qs + W], pqk[:, :],
                            bias[:, oc:oc + 1])
                # V projection: out partitions = seq chunk, cols = proj dims
                for sc in range(4):
                    scg = 4 * q + sc
                    pv = psC.tile([128, OG], F32, tag="c512", name="pv")
                    for kc in range(KC):
                        nc.tensor.matmul(
                            pv[:, :],
                            xT_t[kc][:, sc * 128:(sc + 1) * 128],
                            wv_t[kc][:, :],
                            start=(kc == 0), stop=False)
                    nc.tensor.matmul(pv[:, :], onesb[0:1, :], vb_s[0:1, :],
                                     start=False, stop=True)
                    src = pv.rearrange("p (h c) -> p h c", c=64)
                    dst = v_t[scg].rearrange("p (h c) -> p h c", c=65)
                    nc.vector.tensor_copy(dst[:, :, 0:64], src[:, :, :])

            def phase2_window(w, psA):
                ws = w * W
                chunks = [(c, 0) for c in range(4 * w)]
                chunks += [(4 * w + i, 128 * i) for i in range(4)]
                last = len(chunks) - 1
                for pr in range(NPAIR):
                    he = 2 * pr       # even head (rows 0:64)
                    at_e = psB.tile([128, W], F32, tag="b512", name="at_e")
                    at_o = psB.tile([128, W], F32, tag="b512", name="at_o")
                    for idx, (c, off) in enumerate(chunks):
                        n = W - off
                        sp = psA.tile([128, 2 * W], F32, tag="sp", name="sp")
                        sp3 = sp.rearrange("p (g c) -> p g c", g=2)
                        nc.tensor.matmul(
                            sp[:, off:W],
                            kT_t[pr][0:64, c * 128:(c + 1) * 128],
                            qT_t[pr][0:64, ws + off:ws + W],
                            start=True, stop=True)
                        nc.tensor.matmul(
                            sp[:, W + off:2 * W],
                            kT_t[pr][64:128, c * 128:(c + 1) * 128],
                            qT_t[pr][64:128, ws + off:ws + W],
                            start=True, stop=True)
                        et = p2.tile([128, 2 * W], BF16, tag="E", bufs=6)
                        et3 = et.rearrange("p (g c) -> p g c", g=2)
                        nc.scalar.activation(et3[:, :, off:W],
                                             sp3[:, :, off:W], AF.Exp,
                                             bias=kmask_s[:, c:c + 1],
                                             scale=SCALE)
                        if off or c == 4 * w:  # diagonal chunk
                            nc.vector.tensor_mul(
                                et3[:, :, off:off + 128],
                                et3[:, :, off:off + 128],
                                tri2.rearrange("p (g c) -> p g c", g=2))
                        nc.tensor.matmul(
                            at_e[0:65, off:W],
                            v_t[c][:, he * 65:(he + 1) * 65],
                            et[:, off:W],
                            start=(idx == 0), stop=(idx == last))
                        nc.tensor.matmul(
                            at_o[0:65, off:W],
                            v_t[c][:, (he + 1) * 65:(he + 2) * 65],
                            et[:, W + off:2 * W],
                            start=(idx == 0), stop=(idx == last))
                    # softmax division, off the PE critical path:
                    # evict unnormalized att + denominator row quickly
                    # (frees the PSUM slot), reciprocal on idle GpSimd,
                    # broadcast across partitions via a K=1 matmul.
                    for h, at in ((he, at_e), (he + 1, at_o)):
                        attnU = p2.tile([64, W], BF16, tag="aU", bufs=3)
                        nc.vector.tensor_copy(attnU[:, :], at[0:64, :])
                        dnr = p2.tile([128, W], F32, tag="dnr", bufs=2)
                        nc.vector.tensor_copy(dnr[64:65, :], at[64:65, :])
                        # reciprocal: DVE divide costs 8 cyc per FREE elem,
                        # so reshape the row to [128, 4] via DMA first
                        dnT = p2.tile([128, 4], F32, tag="dnT", bufs=2)
                        nc.sync.dma_start(dnT[:, :], dnr[64:65, :])
                        dnTr = p2.tile([128, 4], BF16, tag="dnTr", bufs=2)
                        with nc.allow_low_precision("recip"):
                            nc.vector.reciprocal(dnTr[:, :], dnT[:, :])
                        rcp = p2.tile([128, W], BF16, tag="rcp", bufs=2)
                        nc.sync.dma_start(rcp[64:65, :], dnTr[:, :])
                        # broadcast the reciprocal row back into the (now
                        # dead) at tile -- reuses its PSUM bank, WAW-ordered
                        # behind the two evictions above
                        nc.tensor.matmul(at[0:64, :], onesb[64:65, 0:64],
                                         rcp[64:65, :], start=True, stop=True)
                        if h % 2 == 0:
                            nc.vector.tensor_mul(
                                attnT_t[pr][0:64, ws:ws + W],
                                attnU[:, :], at[0:64, :])
                        else:
                            tmp = p2.tile([64, W], BF16, tag="tm", bufs=2)
                            nc.vector.tensor_mul(tmp[:, :], attnU[:, :],
                                                 at[0:64, :])
                            nc.sync.dma_start(attnT_t[pr][64:128, ws:ws + W],
                                              tmp[:, :])

            def phase3_window(w, pool, tag):
                # output projection for the sq chunks of window w
                for sc in range(4 * w, 4 * w + 4):
                    ot = p3.tile([128, HID], BF16, tag="ou", bufs=3)
                    for n in range(2):
                        po = pool.tile([128, W], F32, tag=tag, name="po")
                        for kc in range(4):
                            nc.tensor.matmul(
                                po[:, :],
                                attnT_t[kc][:, sc * 128:(sc + 1) * 128],
                                wo_t[kc][:, n * W:(n + 1) * W],
                                start=(kc == 0), stop=(kc == 3))
                        nc.vector.tensor_copy(ot[:, n * W:(n + 1) * W],
                                              po[:, :])
                    nc.sync.dma_start(out_d[sc * 128:(sc + 1) * 128, :],
                                      ot[:, :])

            # interleave projection quarters, attention windows, and output
            # projection so the scheduler can fill PE idle time during
            # ScalarE-bound (softmax) stretches; the last window's output
            # projection runs after the scores pool closes, in a wider pool
            with tc.tile_pool(name="psA", bufs=2, space="PSUM") as psA:
                phase1_quarter(0, x0_t)
                phase2_window(0, psA)
                phase1_quarter(1)
                phase2_window(1, psA)
                phase1_quarter(2)
                phase2_window(2, psA)
                phase3_window(0, psC, "c512")
                phase1_quarter(3)
                phase2_window(3, psA)
                phase3_window(1, psC, "c512")
                phase3_window(2, psC, "c512")
            with tc.tile_pool(name="psD", bufs=4, space="PSUM") as psD:
                phase3_window(3, psD, "d512")

    nc.compile()
    return nc


def kernel(hidden_states, causal_mask, padding_mask,
           q_w, q_b, k_w, k_b, v_w, v_b, o_w, o_b):
    global _compiled
    from concourse.bass_utils import run_bass_kernel_spmd
    import ml_dtypes

    BF = ml_dtypes.bfloat16

    hidden_states = np.asarray(hidden_states, dtype=np.float32)
    padding_mask = np.asarray(padding_mask)
    q_w = np.asarray(q_w, dtype=np.float32)
    k_w = np.asarray(k_w, dtype=np.float32)
    v_w = np.asarray(v_w, dtype=np.float32)
    o_w = np.asarray(o_w, dtype=np.float32)
    q_b = np.asarray(q_b, dtype=np.float32)
    k_b = np.asarray(k_b, dtype=np.float32)
    v_b = np.asarray(v_b, dtype=np.float32)
    o_b = np.asarray(o_b, dtype=np.float32)

    if _compiled is None:
        _compiled = _build()
    nc = _compiled

    in_maps = []
    for b in range(BS):
        xT = np.ascontiguousarray(hidden_states[b].T).astype(BF)
        kmask = np.where(padding_mask[b], np.float32(-30000.0),
                         np.float32(0.0)).astype(np.float32)
        kmask2 = np.ascontiguousarray(kmask.reshape(SC, 128).T)
        for g in range(HG):
            r = slice(g * OG, (g + 1) * OG)
            in_maps.append({
                "xT": xT,
                "wqT": np.ascontiguousarray(q_w[r].T).astype(BF),
                "wkT": np.ascontiguousarray(k_w[r].T).astype(BF),
                "wvT": np.ascontiguousarray(v_w[r].T).astype(BF),
                "woT": np.ascontiguousarray(o_w[:, r].T).astype(BF),
                "qb": np.ascontiguousarray(q_b[r].reshape(4, 128).T),
                "kb": np.ascontiguousarray(k_b[r].reshape(4, 128).T),
                "vb": np.ascontiguousarray(v_b[r].reshape(1, OG)).astype(BF),
                "kmask": kmask2,
            })

    trace = os.environ.get("KERNEL_TRACE") == "1"
    res = run_bass_kernel_spmd(nc, in_maps, core_ids=list(range(NCORES)),
                               trace=trace)
    if trace and res.exec_time_ns is not None:
        print(f"HW exec time: {res.exec_time_ns} ns")
        if res.instructions_and_trace:
            print(f"trace: {res.instructions_and_trace[1]}")

    out = np.empty((BS, SEQ, HID), dtype=np.float32)
    for b in range(BS):
        out[b] = (res.results[2 * b]["out"].astype(np.float32)
                  + res.results[2 * b + 1]["out"].astype(np.float32)
                  + o_b[None, :])
    return out


# revision 39
# speedup vs baseline: 1.2004x; 1.0095x over previous
"""Multi-head attention (bs=4, seq=2048, hidden=1024, 16 heads) on 8 trn2 cores.

Sharding: core = (batch b, head-group g) with 4 batches x 2 groups of 8 heads.
Each core computes QKV projections for its head slice, causal+padded softmax
attention, and a partial output projection; the host sums the two partial
outputs per batch and adds o_b.

v2 layout notes:
  - bf16 weights/activations in SBUF (fp32 accumulate in PSUM); host converts.
  - phase-1 projections run in four 512-seq quarters, emitted interleaved
    with the four 512-query attention windows so the Tile scheduler can keep
    the PE busy with projection matmuls while ScalarE runs softmax exps.
  - score matmuls for a head pair issue to disjoint 64-row PE groups
    (base partitions 0 and 64) so they execute concurrently.
  - exp runs once per (chunk, head-pair) as a single wide ACT op over a
    [128, 2, w] access pattern; padding mask rides as a per-partition bias.
  - softmax division: denominator rows leave PSUM via ScalarE ln, the
    reciprocal is exp(-ln) (same ACT table set), GpSimd broadcasts it
    across partitions, DVE does the final multiply.
"""
import os
import sys

for _p in ("/opt/trn_rl_repo",):
    if _p not in sys.path:
        sys.path.insert(0, _p)

import numpy as np

HID = 1024
HEADS = 16
D = 64
BS = 4
SEQ = 2048
NCORES = 8
HG = 2             # head groups (tensor-parallel axis)
HPG = HEADS // HG  # 8 heads per core
NPAIR = HPG // 2   # 4 head pairs per core
OG = HPG * D       # 512 projection dims per core
KC = HID // 128    # 8 hidden chunks
W = 512            # query window
NW = SEQ // W      # 4 windows (== phase-1 quarters)
SC = SEQ // 128    # 16 key chunks
SCALE = 1.0 / np.sqrt(D)

_compiled = None


def _build():
    import concourse.tile as tile
    from concourse import bacc, mybir

    F32 = mybir.dt.float32
    BF16 = mybir.dt.bfloat16
    AF = mybir.ActivationFunctionType
    Alu = mybir.AluOpType

    nc = bacc.Bacc("TRN2", target_bir_lowering=False, debug=False,
                   num_devices=NCORES)

    xT_d = nc.dram_tensor("xT", [HID, SEQ], BF16, kind="ExternalInput").ap()
    wqT_d = nc.dram_tensor("wqT", [HID, OG], BF16, kind="ExternalInput").ap()
    wkT_d = nc.dram_tensor("wkT", [HID, OG], BF16, kind="ExternalInput").ap()
    wvT_d = nc.dram_tensor("wvT", [HID, OG], BF16, kind="ExternalInput").ap()
    woT_d = nc.dram_tensor("woT", [OG, HID], BF16, kind="ExternalInput").ap()
    qb_d = nc.dram_tensor("qb", [128, 4], F32, kind="ExternalInput").ap()
    kb_d = nc.dram_tensor("kb", [128, 4], F32, kind="ExternalInput").ap()
    vb_d = nc.dram_tensor("vb", [1, OG], BF16, kind="ExternalInput").ap()
    kmask_d = nc.dram_tensor("kmask", [128, SC], F32, kind="ExternalInput").ap()
    out_d = nc.dram_tensor("out", [SEQ, HID], BF16,
                           kind="ExternalOutput").ap()

    with tile.TileContext(nc) as tc:
        with tc.tile_pool(name="const", bufs=1) as cp, \
             tc.tile_pool(name="wq", bufs=1) as wqp, \
             tc.tile_pool(name="wk", bufs=1) as wkp, \
             tc.tile_pool(name="wv", bufs=1) as wvp, \
             tc.tile_pool(name="wo", bufs=1) as wop, \
             tc.tile_pool(name="qT", bufs=1) as qTp, \
             tc.tile_pool(name="kT", bufs=1) as kTp, \
             tc.tile_pool(name="v", bufs=1) as vp, \
             tc.tile_pool(name="attnT", bufs=1) as aTp, \
             tc.tile_pool(name="x", bufs=2) as xp, \
             tc.tile_pool(name="ph2", bufs=1) as p2, \
             tc.tile_pool(name="ph3", bufs=1) as p3, \
             tc.tile_pool(name="psB", bufs=3, space="PSUM") as psB, \
             tc.tile_pool(name="psC", bufs=1, space="PSUM") as psC:

            # ---------------- constants ----------------
            ones_f = cp.tile([128, 128], F32, tag="ones_f")
            nc.gpsimd.memset(ones_f[:, :], 1.0)
            onesb = cp.tile([128, 128], BF16, tag="onesb")
            nc.scalar.copy(onesb[:, :], ones_f[:, :])
            # tri01[p, j] = 1 if j >= p else 0 (keep keys <= query), two
            # adjacent copies so one 3D-AP multiply masks both heads.
            tri01_f = cp.tile([128, 128], F32, tag="tri01_f")
            nc.gpsimd.affine_select(tri01_f[:, :], ones_f[:, :],
                                    pattern=[[1, 128]],
                                    compare_op=Alu.is_ge, fill=0.0,
                                    base=0, channel_multiplier=-1)
            tri2 = cp.tile([128, 256], BF16, tag="tri2")
            nc.scalar.copy(tri2[:, 0:128], tri01_f[:, :])
            nc.scalar.copy(tri2[:, 128:256], tri01_f[:, :])
            qb_s = cp.tile([128, 4], F32, tag="qb")
            nc.sync.dma_start(qb_s[:, :], qb_d[:, :])
            kb_s = cp.tile([128, 4], F32, tag="kb")
            nc.sync.dma_start(kb_s[:, :], kb_d[:, :])
            vb_s = cp.tile([1, OG], BF16, tag="vb")
            nc.sync.dma_start(vb_s[:, :], vb_d[:, :])
            kmask_s = cp.tile([128, SC], F32, tag="km")
            nc.sync.dma_start(kmask_s[:, :], kmask_d[:, :])

            def load_x_quarter(q):
                # one batched DMA for the whole quarter (the Sync queue
                # costs ~600ns per dma_start issue)
                qs = q * W
                xf = xp.tile([128, KC * W], BF16, tag="xTf", name=f"xT{q}")
                nc.sync.dma_start(
                    xf[:, :].rearrange("p (kc s) -> p kc s", kc=KC),
                    xT_d[:, qs:qs + W].rearrange("(kc p) s -> p kc s", p=128))
                return [xf[:, kc * W:(kc + 1) * W] for kc in range(KC)]

            # quarter-0 activations first so the first projection matmuls
            # can start while the bulk of the weights still stream in
            x0_t = load_x_quarter(0)

            # ---------------- weights (one batched DMA each) ----------------
            def load_wflat(pool, src, nkc, width, name):
                flat = pool.tile([128, nkc * width], BF16, tag=name)
                nc.sync.dma_start(
                    flat[:, :].rearrange("p (kc s) -> p kc s", kc=nkc),
                    src.rearrange("(kc p) s -> p kc s", p=128))
                return [flat[:, kc * width:(kc + 1) * width]
                        for kc in range(nkc)]

            wq_t = load_wflat(wqp, wqT_d, KC, OG, "wqf")
            wk_t = load_wflat(wkp, wkT_d, KC, OG, "wkf")
            wv_t = load_wflat(wvp, wvT_d, KC, OG, "wvf")
            wo_t = load_wflat(wop, woT_d, 4, HID, "wof")

            # ---------------- persistent activation tiles ----------------
            qT_t = [qTp.tile([128, SEQ], BF16, tag=f"qT{i}", name=f"qT{i}")
                    for i in range(NPAIR)]
            kT_t = [kTp.tile([128, SEQ], BF16, tag=f"kT{i}", name=f"kT{i}")
                    for i in range(NPAIR)]
            v_t = [vp.tile([128, HPG * 65], BF16, tag=f"v{i}", name=f"v{i}")
                   for i in range(SC)]
            for i in range(SC):
                vv = v_t[i].rearrange("p (h c) -> p h c", c=65)
                nc.gpsimd.memset(vv[:, :, 64:65], 1.0)
            attnT_t = [aTp.tile([128, SEQ], BF16, tag=f"aT{i}", name=f"aT{i}")
                       for i in range(NPAIR)]

            def phase1_quarter(q, xT_t=None):
                qs = q * W
                if xT_t is None:
                    xT_t = load_x_quarter(q)
                # Q/K projections: out partitions = proj dims, cols = seq
                for w_t, o_t, bias in ((wq_t, qT_t, qb_s), (wk_t, kT_t, kb_s)):
                    for oc in range(4):
                        pqk = psC.tile([128, W], F32, tag="c512", name="pqk")
                        for kc in range(KC):
                            nc.tensor.matmul(
                                pqk[:, :],
                                w_t[kc][:, oc * 128:(oc + 1) * 128],
                                xT_t[kc][:, :],
                                start=(kc == 0), stop=(kc == KC - 1))
                        nc.vector.tensor_scalar_add(
                            o_t[oc][:, qs:qs + W], pqk[:, :],
                            bias[:, oc:oc + 1])
                # V projection: out partitions = seq chunk, cols = proj dims
                for sc in range(4):
                    scg = 4 * q + sc
                    pv = psC.tile([128, OG], F32, tag="c512", name="pv")
                    for kc in range(KC):
                        nc.tensor.matmul(
                            pv[:, :],
                            xT_t[kc][:, sc * 128:(sc + 1) * 128],
                            wv_t[kc][:, :],
                            start=(kc == 0), stop=False)
                    nc.tensor.matmul(pv[:, :], onesb[0:1, :], vb_s[0:1, :],
                                     start=False, stop=True)
                    src = pv.rearrange("p (h c) -> p h c", c=64)
                    dst = v_t[scg].rearrange("p (h c) -> p h c", c=65)
                    nc.vector.tensor_copy(dst[:, :, 0:64], src[:, :, :])

            def phase2_window(w, psA):
                ws = w * W
                chunks = [(c, 0) for c in range(4 * w)]
                chunks += [(4 * w + i, 128 * i) for i in range(4)]
                last = len(chunks) - 1
                for pr in range(NPAIR):
                    he = 2 * pr       # even head (rows 0:64)
                    at_e = psB.tile([128, W], F32, tag="b512", name="at_e")
                    at_o = psB.tile([128, W], F32, tag="b512", name="at_o")
                    for idx, (c, off) in enumerate(chunks):
                        n = W - off
                        sp = psA.tile([128, 2 * W], F32, tag="sp", name="sp")
                        sp3 = sp.rearrange("p (g c) -> p g c", g=2)
                        nc.tensor.matmul(
                            sp[:, off:W],
                            kT_t[pr][0:64, c * 128:(c + 1) * 128],
                            qT_t[pr][0:64, ws + off:ws + W],
                            start=True, stop=True)
                        nc.tensor.matmul(
                            sp[:, W + off:2 * W],
                            kT_t[pr][64:128, c * 128:(c + 1) * 128],
                            qT_t[pr][64:128, ws + off:ws + W],
                            start=True, stop=True)
                        et = p2.tile([128, 2 * W], BF16, tag="E", bufs=6)
                        et3 = et.rearrange("p (g c) -> p g c", g=2)
                        nc.scalar.activation(et3[:, :, off:W],
                                             sp3[:, :, off:W], AF.Exp,
                                             bias=kmask_s[:, c:c + 1],
                                             scale=SCALE)
                        if off or c == 4 * w:  # diagonal chunk
                            nc.vector.tensor_mul(
                                et3[:, :, off:off + 128],
                                et3[:, :, off:off + 128],
                                tri2.rearrange("p (g c) -> p g c", g=2))
                        nc.tensor.matmul(
                            at_e[0:65, off:W],
                            v_t[c][:, he * 65:(he + 1) * 65],
                            et[:, off:W],
                            start=(idx == 0), stop=(idx == last))
                        nc.tensor.matmul(
                            at_o[0:65, off:W],
                            v_t[c][:, (he + 1) * 65:(he + 2) * 65],
                            et[:, W + off:2 * W],
                            start=(idx == 0), stop=(idx == last))
                    # softmax division, off the PE critical path:
                    # evict unnormalized att + denominator row quickly
                    # (frees the PSUM slot), reciprocal on idle GpSimd,
                    # broadcast across partitions via a K=1 matmul.
                    for h, at in ((he, at_e), (he + 1, at_o)):
                        attnU = p2.tile([64, W], BF16, tag="aU", bufs=3)
                        nc.vector.tensor_copy(attnU[:, :], at[0:64, :])
                        dnr = p2.tile([128, W], F32, tag="dnr", bufs=2)
                        nc.vector.tensor_copy(dnr[64:65, :], at[64:65, :])
                        # reciprocal: DVE divide costs 8 cyc per FREE elem,
                        # so reshape the row to [128, 4] via DMA first
                        dnT = p2.tile([128, 4], F32, tag="dnT", bufs=2)
                        nc.sync.dma_start(dnT[:, :], dnr[64:65, :])
                        dnTr = p2.tile([128, 4], BF16, tag="dnTr", bufs=2)
                        with nc.allow_low_precision("recip"):
                            nc.vector.reciprocal(dnTr[:, :], dnT[:, :])
                        rcp = p2.tile([128, W], BF16, tag="rcp", bufs=2)
                        nc.sync.dma_start(rcp[64:65, :], dnTr[:, :])
                        # broadcast the reciprocal row back into the (now
                        # dead) at tile -- reuses its PSUM bank, WAW-ordered
                        # behind the two evictions above
                        nc.tensor.matmul(at[0:64, :], onesb[64:65, 0:64],
                                         rcp[64:65, :], start=True, stop=True)
                        if h % 2 == 0:
                            nc.vector.tensor_mul(
                                attnT_t[pr][0:64, ws:ws + W],
                                attnU[:, :], at[0:64, :])
                        else:
                            tmp = p2.tile([64, W], BF16, tag="tm", bufs=2)
                            nc.vector.tensor_mul(tmp[:, :], attnU[:, :],
                                                 at[0:64, :])
                            nc.sync.dma_start(attnT_t[pr][64:128, ws:ws + W],
                                              tmp[:, :])

            def phase3_window(w, pool, tag):
                # output projection for the sq chunks of window w
                for sc in range(4 * w, 4 * w + 4):
                    ot = p3.tile([128, HID], BF16, tag="ou", bufs=3)
                    for n in range(2):
                        po = pool.tile([128, W], F32, tag=tag, name="po")
                        for kc in range(4):
                            nc.tensor.matmul(
                                po[:, :],
                                attnT_t[kc][:, sc * 128:(sc + 1) * 128],
                                wo_t[kc][:, n * W:(n + 1) * W],
                                start=(kc == 0), stop=(kc == 3))
                        nc.vector.tensor_copy(ot[:, n * W:(n + 1) * W],
                                              po[:, :])
                    nc.sync.dma_start(out_d[sc * 128:(sc + 1) * 128, :],
                                      ot[:, :])

            # interleave projection quarters, attention windows, and output
            # projection so the scheduler can fill PE idle time during
            # ScalarE-bound (softmax) stretches; the last window's output
            # projection runs after the scores pool closes, in a wider pool
            with tc.tile_pool(name="psA", bufs=2, space="PSUM") as psA:
                phase1_quarter(0, x0_t)
                phase2_window(0, psA)
                phase1_quarter(1)
                phase2_window(1, psA)
                phase1_quarter(2)
                phase2_window(2, psA)
                phase3_window(0, psC, "c512")
                phase1_quarter(3)
                phase2_window(3, psA)
                phase3_window(1, psC, "c512")
                phase3_window(2, psC, "c512")
            with tc.tile_pool(name="psD", bufs=4, space="PSUM") as psD:
                phase3_window(3, psD, "d512")

    nc.compile()
    return nc


def kernel(hidden_states, causal_mask, padding_mask,
           q_w, q_b, k_w, k_b, v_w, v_b, o_w, o_b):
    global _compiled
    from concourse.bass_utils import run_bass_kernel_spmd
    import ml_dtypes

    BF = ml_dtypes.bfloat16

    hidden_states = np.asarray(hidden_states, dtype=np.float32)
    padding_mask = np.asarray(padding_mask)
    q_w = np.asarray(q_w, dtype=np.float32)
    k_w = np.asarray(k_w, dtype=np.float32)
    v_w = np.asarray(v_w, dtype=np.float32)
    o_w = np.asarray(o_w, dtype=np.float32)
    q_b = np.asarray(q_b, dtype=np.float32)
    k_b = np.asarray(k_b, dtype=np.float32)
    v_b = np.asarray(v_b, dtype=np.float32)
    o_b = np.asarray(o_b, dtype=np.float32)

    if _compiled is None:
        _compiled = _build()
    nc = _compiled

    in_maps = []
    for b in range(BS):
        xT = np.ascontiguousarray(hidden_states[b].T).astype(BF)
        kmask = np.where(padding_mask[b], np.float32(-30000.0),
                         np.float32(0.0)).astype(np.float32)
        kmask2 = np.ascontiguousarray(kmask.reshape(SC, 128).T)
        for g in range(HG):
            r = slice(g * OG, (g + 1) * OG)
            in_maps.append({
                "xT": xT,
                "wqT": np.ascontiguousarray(q_w[r].T).astype(BF),
                "wkT": np.ascontiguousarray(k_w[r].T).astype(BF),
                "wvT": np.ascontiguousarray(v_w[r].T).astype(BF),
                "woT": np.ascontiguousarray(o_w[:, r].T).astype(BF),
                "qb": np.ascontiguousarray(q_b[r].reshape(4, 128).T),
                "kb": np.ascontiguousarray(k_b[r].reshape(4, 128).T),
                "vb": np.ascontiguousarray(v_b[r].reshape(1, OG)).astype(BF),
                "kmask": kmask2,
            })

    trace = os.environ.get("KERNEL_TRACE") == "1"
    res = run_bass_kernel_spmd(nc, in_maps, core_ids=list(range(NCORES)),
                               trace=trace)
    if trace and res.exec_time_ns is not None:
        print(f"HW exec time: {res.exec_time_ns} ns")
        if res.instructions_and_trace:
            print(f"trace: {res.instructions_and_trace[1]}")

    out = np.empty((BS, SEQ, HID), dtype=np.float32)
    for b in range(BS):
        out[b] = (res.results[2 * b]["out"].astype(np.float32)
                  + res.results[2 * b + 1]["out"].astype(np.float32)
                  + o_b[None, :])
    return out


# revision 40
# speedup vs baseline: 1.2128x; 1.0104x over previous
"""Multi-head attention (bs=4, seq=2048, hidden=1024, 16 heads) on 8 trn2 cores.

Sharding: core = (batch b, head-group g) with 4 batches x 2 groups of 8 heads.
Each core computes QKV projections for its head slice, causal+padded softmax
attention, and a partial output projection; the host sums the two partial
outputs per batch and adds o_b.

v2 layout notes:
  - bf16 weights/activations in SBUF (fp32 accumulate in PSUM); host converts.
  - phase-1 projections run in four 512-seq quarters, emitted interleaved
    with the four 512-query attention windows so the Tile scheduler can keep
    the PE busy with projection matmuls while ScalarE runs softmax exps.
  - score matmuls for a head pair issue to disjoint 64-row PE groups
    (base partitions 0 and 64) so they execute concurrently.
  - exp runs once per (chunk, head-pair) as a single wide ACT op over a
    [128, 2, w] access pattern; padding mask rides as a per-partition bias.
  - softmax division: denominator rows leave PSUM via ScalarE ln, the
    reciprocal is exp(-ln) (same ACT table set), GpSimd broadcasts it
    across partitions, DVE does the final multiply.
"""
import os
import sys

for _p in ("/opt/trn_rl_repo",):
    if _p not in sys.path:
        sys.path.insert(0, _p)

import numpy as np

HID = 1024
HEADS = 16
D = 64
BS = 4
SEQ = 2048
NCORES = 8
HG = 2             # head groups (tensor-parallel axis)
HPG = HEADS // HG  # 8 heads per core
NPAIR = HPG // 2   # 4 head pairs per core
OG = HPG * D       # 512 projection dims per core
KC = HID // 128    # 8 hidden chunks
W = 512            # query window
NW = SEQ // W      # 4 windows (== phase-1 quarters)
SC = SEQ // 128    # 16 key chunks
SCALE = 1.0 / np.sqrt(D)

_compiled = None


def _build():
    import concourse.tile as tile
    from concourse import bacc, mybir

    F32 = mybir.dt.float32
    BF16 = mybir.dt.bfloat16
    AF = mybir.ActivationFunctionType
    Alu = mybir.AluOpType

    nc = bacc.Bacc("TRN2", target_bir_lowering=False, debug=False,
                   num_devices=NCORES)

    xT_d = nc.dram_tensor("xT", [HID, SEQ], BF16, kind="ExternalInput").ap()
    wqT_d = nc.dram_tensor("wqT", [HID, OG], BF16, kind="ExternalInput").ap()
    wkT_d = nc.dram_tensor("wkT", [HID, OG], BF16, kind="ExternalInput").ap()
    wvT_d = nc.dram_tensor("wvT", [HID, OG], BF16, kind="ExternalInput").ap()
    woT_d = nc.dram_tensor("woT", [OG, HID], BF16, kind="ExternalInput").ap()
    qb_d = nc.dram_tensor("qb", [128, 4], F32, kind="ExternalInput").ap()
    kb_d = nc.dram_tensor("kb", [128, 4], F32, kind="ExternalInput").ap()
    vb_d = nc.dram_tensor("vb", [1, OG], BF16, kind="ExternalInput").ap()
    kmask_d = nc.dram_tensor("kmask", [128, SC], F32, kind="ExternalInput").ap()
    out_d = nc.dram_tensor("out", [SEQ, HID], BF16,
                           kind="ExternalOutput").ap()

    with tile.TileContext(nc) as tc:
        with tc.tile_pool(name="const", bufs=1) as cp, \
             tc.tile_pool(name="wq", bufs=1) as wqp, \
             tc.tile_pool(name="wk", bufs=1) as wkp, \
             tc.tile_pool(name="wv", bufs=1) as wvp, \
             tc.tile_pool(name="wo", bufs=1) as wop, \
             tc.tile_pool(name="qT", bufs=1) as qTp, \
             tc.tile_pool(name="kT", bufs=1) as kTp, \
             tc.tile_pool(name="v", bufs=1) as vp, \
             tc.tile_pool(name="attnT", bufs=1) as aTp, \
             tc.tile_pool(name="x", bufs=3) as xp, \
             tc.tile_pool(name="ph2", bufs=1) as p2, \
             tc.tile_pool(name="ph3", bufs=1) as p3, \
             tc.tile_pool(name="psB", bufs=3, space="PSUM") as psB, \
             tc.tile_pool(name="psC", bufs=1, space="PSUM") as psC:

            # ---------------- constants ----------------
            ones_f = cp.tile([128, 128], F32, tag="ones_f")
            nc.gpsimd.memset(ones_f[:, :], 1.0)
            onesb = cp.tile([128, 128], BF16, tag="onesb")
            nc.scalar.copy(onesb[:, :], ones_f[:, :])
            # tri01[p, j] = 1 if j >= p else 0 (keep keys <= query), two
            # adjacent copies so one 3D-AP multiply masks both heads.
            tri01_f = cp.tile([128, 128], F32, tag="tri01_f")
            nc.gpsimd.affine_select(tri01_f[:, :], ones_f[:, :],
                                    pattern=[[1, 128]],
                                    compare_op=Alu.is_ge, fill=0.0,
                                    base=0, channel_multiplier=-1)
            tri2 = cp.tile([128, 256], BF16, tag="tri2")
            nc.scalar.copy(tri2[:, 0:128], tri01_f[:, :])
            nc.scalar.copy(tri2[:, 128:256], tri01_f[:, :])
            qb_s = cp.tile([128, 4], F32, tag="qb")
            nc.sync.dma_start(qb_s[:, :], qb_d[:, :])
            kb_s = cp.tile([128, 4], F32, tag="kb")
            nc.sync.dma_start(kb_s[:, :], kb_d[:, :])
            vb_s = cp.tile([1, OG], BF16, tag="vb")
            nc.sync.dma_start(vb_s[:, :], vb_d[:, :])
            kmask_s = cp.tile([128, SC], F32, tag="km")
            nc.sync.dma_start(kmask_s[:, :], kmask_d[:, :])

            def load_x_quarter(q):
                # one batched DMA for the whole quarter (the Sync queue
                # costs ~600ns per dma_start issue)
                qs = q * W
                xf = xp.tile([128, KC * W], BF16, tag="xTf", name=f"xT{q}")
                nc.sync.dma_start(
                    xf[:, :].rearrange("p (kc s) -> p kc s", kc=KC),
                    xT_d[:, qs:qs + W].rearrange("(kc p) s -> p kc s", p=128))
                return [xf[:, kc * W:(kc + 1) * W] for kc in range(KC)]

            # quarter-0 activations first so the first projection matmuls
            # can start while the bulk of the weights still stream in
            x0_t = load_x_quarter(0)

            # ---------------- weights (one batched DMA each) ----------------
            def load_wflat(pool, src, nkc, width, name):
                flat = pool.tile([128, nkc * width], BF16, tag=name)
                nc.sync.dma_start(
                    flat[:, :].rearrange("p (kc s) -> p kc s", kc=nkc),
                    src.rearrange("(kc p) s -> p kc s", p=128))
                return [flat[:, kc * width:(kc + 1) * width]
                        for kc in range(nkc)]

            wq_t = load_wflat(wqp, wqT_d, KC, OG, "wqf")
            wk_t = load_wflat(wkp, wkT_d, KC, OG, "wkf")
            wv_t = load_wflat(wvp, wvT_d, KC, OG, "wvf")
            wo_t = load_wflat(wop, woT_d, 4, HID, "wof")

            # ---------------- persistent activation tiles ----------------
            qT_t = [qTp.tile([128, SEQ], BF16, tag=f"qT{i}", name=f"qT{i}")
                    for i in range(NPAIR)]
            kT_t = [kTp.tile([128, SEQ], BF16, tag=f"kT{i}", name=f"kT{i}")
                    for i in range(NPAIR)]
            v_t = [vp.tile([128, HPG * 65], BF16, tag=f"v{i}", name=f"v{i}")
                   for i in range(SC)]
            for i in range(SC):
                vv = v_t[i].rearrange("p (h c) -> p h c", c=65)
                nc.gpsimd.memset(vv[:, :, 64:65], 1.0)
            attnT_t = [aTp.tile([128, SEQ], BF16, tag=f"aT{i}", name=f"aT{i}")
                       for i in range(NPAIR)]

            def phase1_quarter(q, xT_t=None):
                qs = q * W
                if xT_t is None:
                    xT_t = load_x_quarter(q)
                # Q/K projections: out partitions = proj dims, cols = seq
                for w_t, o_t, bias in ((wq_t, qT_t, qb_s), (wk_t, kT_t, kb_s)):
                    for oc in range(4):
                        pqk = psC.tile([128, W], F32, tag="c512", name="pqk")
                        for kc in range(KC):
                            nc.tensor.matmul(
                                pqk[:, :],
                                w_t[kc][:, oc * 128:(oc + 1) * 128],
                                xT_t[kc][:, :],
                                start=(kc == 0), stop=(kc == KC - 1))
                        nc.vector.tensor_scalar_add(
                            o_t[oc][:, qs:qs + W], pqk[:, :],
                            bias[:, oc:oc + 1])
                # V projection: out partitions = seq chunk, cols = proj dims
                for sc in range(4):
                    scg = 4 * q + sc
                    pv = psC.tile([128, OG], F32, tag="c512", name="pv")
                    for kc in range(KC):
                        nc.tensor.matmul(
                            pv[:, :],
                            xT_t[kc][:, sc * 128:(sc + 1) * 128],
                            wv_t[kc][:, :],
                            start=(kc == 0), stop=False)
                    nc.tensor.matmul(pv[:, :], onesb[0:1, :], vb_s[0:1, :],
                                     start=False, stop=True)
                    src = pv.rearrange("p (h c) -> p h c", c=64)
                    dst = v_t[scg].rearrange("p (h c) -> p h c", c=65)
                    nc.vector.tensor_copy(dst[:, :, 0:64], src[:, :, :])

            def phase2_window(w, psA):
                ws = w * W
                chunks = [(c, 0) for c in range(4 * w)]
                chunks += [(4 * w + i, 128 * i) for i in range(4)]
                last = len(chunks) - 1
                for pr in range(NPAIR):
                    he = 2 * pr       # even head (rows 0:64)
                    at_e = psB.tile([128, W], F32, tag="b512", name="at_e")
                    at_o = psB.tile([128, W], F32, tag="b512", name="at_o")
                    for idx, (c, off) in enumerate(chunks):
                        n = W - off
                        sp = psA.tile([128, 2 * W], F32, tag="sp", name="sp")
                        sp3 = sp.rearrange("p (g c) -> p g c", g=2)
                        nc.tensor.matmul(
                            sp[:, off:W],
                            kT_t[pr][0:64, c * 128:(c + 1) * 128],
                            qT_t[pr][0:64, ws + off:ws + W],
                            start=True, stop=True)
                        nc.tensor.matmul(
                            sp[:, W + off:2 * W],
                            kT_t[pr][64:128, c * 128:(c + 1) * 128],
                            qT_t[pr][64:128, ws + off:ws + W],
                            start=True, stop=True)
                        et = p2.tile([128, 2 * W], BF16, tag="E", bufs=8)
                        et3 = et.rearrange("p (g c) -> p g c", g=2)
                        nc.scalar.activation(et3[:, :, off:W],
                                             sp3[:, :, off:W], AF.Exp,
                                             bias=kmask_s[:, c:c + 1],
                                             scale=SCALE)
                        if off or c == 4 * w:  # diagonal chunk
                            nc.vector.tensor_mul(
                                et3[:, :, off:off + 128],
                                et3[:, :, off:off + 128],
                                tri2.rearrange("p (g c) -> p g c", g=2))
                        nc.tensor.matmul(
                            at_e[0:65, off:W],
                            v_t[c][:, he * 65:(he + 1) * 65],
                            et[:, off:W],
                            start=(idx == 0), stop=(idx == last))
                        nc.tensor.matmul(
                            at_o[0:65, off:W],
                            v_t[c][:, (he + 1) * 65:(he + 2) * 65],
                            et[:, W + off:2 * W],
                            start=(idx == 0), stop=(idx == last))
                    # softmax division, off the PE critical path:
                    # evict unnormalized att + denominator row quickly
                    # (frees the PSUM slot), reciprocal on idle GpSimd,
                    # broadcast across partitions via a K=1 matmul.
                    for h, at in ((he, at_e), (he + 1, at_o)):
                        attnU = p2.tile([64, W], BF16, tag="aU", bufs=4)
                        nc.vector.tensor_copy(attnU[:, :], at[0:64, :])
                        dnr = p2.tile([128, W], F32, tag="dnr", bufs=2)
                        nc.vector.tensor_copy(dnr[64:65, :], at[64:65, :])
                        # reciprocal: DVE divide costs 8 cyc per FREE elem,
                        # so reshape the row to [128, 4] via DMA first
                        dnT = p2.tile([128, 4], F32, tag="dnT", bufs=2)
                        nc.sync.dma_start(dnT[:, :], dnr[64:65, :])
                        dnTr = p2.tile([128, 4], BF16, tag="dnTr", bufs=2)
                        with nc.allow_low_precision("recip"):
                            nc.vector.reciprocal(dnTr[:, :], dnT[:, :])
                        rcp = p2.tile([128, W], BF16, tag="rcp", bufs=2)
                        nc.sync.dma_start(rcp[64:65, :], dnTr[:, :])
                        # broadcast the reciprocal row back into the (now
                        # dead) at tile -- reuses its PSUM bank, WAW-ordered
                        # behind the two evictions above
                        nc.tensor.matmul(at[0:64, :], onesb[64:65, 0:64],
                                         rcp[64:65, :], start=True, stop=True)
                        if h % 2 == 0:
                            nc.vector.tensor_mul(
                                attnT_t[pr][0:64, ws:ws + W],
                                attnU[:, :], at[0:64, :])
                        else:
                            tmp = p2.tile([64, W], BF16, tag="tm", bufs=3)
                            nc.vector.tensor_mul(tmp[:, :], attnU[:, :],
                                                 at[0:64, :])
                            nc.sync.dma_start(attnT_t[pr][64:128, ws:ws + W],
                                              tmp[:, :])

            def phase3_window(w, pool, tag):
                # output projection for the sq chunks of window w
                for sc in range(4 * w, 4 * w + 4):
                    ot = p3.tile([128, HID], BF16, tag="ou", bufs=3)
                    for n in range(2):
                        po = pool.tile([128, W], F32, tag=tag, name="po")
                        for kc in range(4):
                            nc.tensor.matmul(
                                po[:, :],
                                attnT_t[kc][:, sc * 128:(sc + 1) * 128],
                                wo_t[kc][:, n * W:(n + 1) * W],
                                start=(kc == 0), stop=(kc == 3))
                        nc.vector.tensor_copy(ot[:, n * W:(n + 1) * W],
                                              po[:, :])
                    nc.sync.dma_start(out_d[sc * 128:(sc + 1) * 128, :],
                                      ot[:, :])

            # interleave projection quarters, attention windows, and output
            # projection so the scheduler can fill PE idle time during
            # ScalarE-bound (softmax) stretches; the last window's output
            # projection runs after the scores pool closes, in a wider pool
            with tc.tile_pool(name="psA", bufs=2, space="PSUM") as psA:
                phase1_quarter(0, x0_t)
                phase2_window(0, psA)
                phase1_quarter(1)
                phase2_window(1, psA)
                phase1_quarter(2)
                phase2_window(2, psA)
                phase3_window(0, psC, "c512")
                phase1_quarter(3)
                phase2_window(3, psA)
                phase3_window(1, psC, "c512")
                phase3_window(2, psC, "c512")
            with tc.tile_pool(name="psD", bufs=4, space="PSUM") as psD:
                phase3_window(3, psD, "d512")

    nc.compile()
    return nc


def kernel(hidden_states, causal_mask, padding_mask,
           q_w, q_b, k_w, k_b, v_w, v_b, o_w, o_b):
    global _compiled
    from concourse.bass_utils import run_bass_kernel_spmd
    import ml_dtypes

    BF = ml_dtypes.bfloat16

    hidden_states = np.asarray(hidden_states, dtype=np.float32)
    padding_mask = np.asarray(padding_mask)
    q_w = np.asarray(q_w, dtype=np.float32)
    k_w = np.asarray(k_w, dtype=np.float32)
    v_w = np.asarray(v_w, dtype=np.float32)
    o_w = np.asarray(o_w, dtype=np.float32)
    q_b = np.asarray(q_b, dtype=np.float32)
    k_b = np.asarray(k_b, dtype=np.float32)
    v_b = np.asarray(v_b, dtype=np.float32)
    o_b = np.asarray(o_b, dtype=np.float32)

    if _compiled is None:
        _compiled = _build()
    nc = _compiled

    in_maps = []
    for b in range(BS):
        xT = np.ascontiguousarray(hidden_states[b].T).astype(BF)
        kmask = np.where(padding_mask[b], np.float32(-30000.0),
                         np.float32(0.0)).astype(np.float32)
        kmask2 = np.ascontiguousarray(kmask.reshape(SC, 128).T)
        for g in range(HG):
            r = slice(g * OG, (g + 1) * OG)
            in_maps.append({
                "xT": xT,
                "wqT": np.ascontiguousarray(q_w[r].T).astype(BF),
                "wkT": np.ascontiguousarray(k_w[r].T).astype(BF),
                "wvT": np.ascontiguousarray(v_w[r].T).astype(BF),
                "woT": np.ascontiguousarray(o_w[:, r].T).astype(BF),
                "qb": np.ascontiguousarray(q_b[r].reshape(4, 128).T),
                "kb": np.ascontiguousarray(k_b[r].reshape(4, 128).T),
                "vb": np.ascontiguousarray(v_b[r].reshape(1, OG)).astype(BF),
                "kmask": kmask2,
            })

    trace = os.environ.get("KERNEL_TRACE") == "1"
    res = run_bass_kernel_spmd(nc, in_maps, core_ids=list(range(NCORES)),
                               trace=trace)
    if trace and res.exec_time_ns is not None:
        print(f"HW exec time: {res.exec_time_ns} ns")
        if res.instructions_and_trace:
            print(f"trace: {res.instructions_and_trace[1]}")

    out = np.empty((BS, SEQ, HID), dtype=np.float32)
    for b in range(BS):
        out[b] = (res.results[2 * b]["out"].astype(np.float32)
                  + res.results[2 * b + 1]["out"].astype(np.float32)
                  + o_b[None, :])
    return out


# revision 43
# speedup vs baseline: 1.2455x; 1.0269x over previous
"""Multi-head attention (bs=4, seq=2048, hidden=1024, 16 heads) on 8 trn2 cores.

Sharding: core = (batch b, head-group g) with 4 batches x 2 groups of 8 heads.
Each core computes QKV projections for its head slice, causal+padded softmax
attention, and a partial output projection; the host sums the two partial
outputs per batch and adds o_b.

v2 layout notes:
  - bf16 weights/activations in SBUF (fp32 accumulate in PSUM); host converts.
  - phase-1 projections run in four 512-seq quarters, emitted interleaved
    with the four 512-query attention windows so the Tile scheduler can keep
    the PE busy with projection matmuls while ScalarE runs softmax exps.
  - score matmuls for a head pair issue to disjoint 64-row PE groups
    (base partitions 0 and 64) so they execute concurrently.
  - exp runs once per (chunk, head-pair) as a single wide ACT op over a
    [128, 2, w] access pattern; padding mask rides as a per-partition bias.
  - softmax division: denominator rows leave PSUM via ScalarE ln, the
    reciprocal is exp(-ln) (same ACT table set), GpSimd broadcasts it
    across partitions, DVE does the final multiply.
"""
import os
import sys

for _p in ("/opt/trn_rl_repo",):
    if _p not in sys.path:
        sys.path.insert(0, _p)

import numpy as np

HID = 1024
HEADS = 16
D = 64
BS = 4
SEQ = 2048
NCORES = 8
HG = 2             # head groups (tensor-parallel axis)
HPG = HEADS // HG  # 8 heads per core
NPAIR = HPG // 2   # 4 head pairs per core
OG = HPG * D       # 512 projection dims per core
KC = HID // 128    # 8 hidden chunks
W = 512            # query window
NW = SEQ // W      # 4 windows (== phase-1 quarters)
SC = SEQ // 128    # 16 key chunks
SCALE = 1.0 / np.sqrt(D)

_compiled = None


def _build():
    import concourse.tile as tile
    from concourse import bacc, mybir

    F32 = mybir.dt.float32
    BF16 = mybir.dt.bfloat16
    AF = mybir.ActivationFunctionType
    Alu = mybir.AluOpType

    nc = bacc.Bacc("TRN2", target_bir_lowering=False, debug=False,
                   num_devices=NCORES)

    xT_d = nc.dram_tensor("xT", [HID, SEQ], BF16, kind="ExternalInput").ap()
    wqT_d = nc.dram_tensor("wqT", [HID, OG], BF16, kind="ExternalInput").ap()
    wkT_d = nc.dram_tensor("wkT", [HID, OG], BF16, kind="ExternalInput").ap()
    wvT_d = nc.dram_tensor("wvT", [HID, OG], BF16, kind="ExternalInput").ap()
    woT_d = nc.dram_tensor("woT", [OG, HID], BF16, kind="ExternalInput").ap()
    qb_d = nc.dram_tensor("qb", [128, 4], F32, kind="ExternalInput").ap()
    kb_d = nc.dram_tensor("kb", [128, 4], F32, kind="ExternalInput").ap()
    vb_d = nc.dram_tensor("vb", [1, OG], BF16, kind="ExternalInput").ap()
    kmask_d = nc.dram_tensor("kmask", [128, SC], F32, kind="ExternalInput").ap()
    out_d = nc.dram_tensor("out", [SEQ, HID], BF16,
                           kind="ExternalOutput").ap()

    with tile.TileContext(nc) as tc:
        with tc.tile_pool(name="const", bufs=1) as cp, \
             tc.tile_pool(name="wq", bufs=1) as wqp, \
             tc.tile_pool(name="wk", bufs=1) as wkp, \
             tc.tile_pool(name="wv", bufs=1) as wvp, \
             tc.tile_pool(name="wo", bufs=1) as wop, \
             tc.tile_pool(name="qT", bufs=1) as qTp, \
             tc.tile_pool(name="kT", bufs=1) as kTp, \
             tc.tile_pool(name="v", bufs=1) as vp, \
             tc.tile_pool(name="attnT", bufs=1) as aTp, \
             tc.tile_pool(name="x", bufs=3) as xp, \
             tc.tile_pool(name="ph2", bufs=1) as p2, \
             tc.tile_pool(name="ph3", bufs=1) as p3, \
             tc.tile_pool(name="psB", bufs=3, space="PSUM") as psB, \
             tc.tile_pool(name="psC", bufs=1, space="PSUM") as psC:

            # ---------------- constants ----------------
            ones_f = cp.tile([128, 128], F32, tag="ones_f")
            nc.gpsimd.memset(ones_f[:, :], 1.0)
            onesb = cp.tile([128, 128], BF16, tag="onesb")
            nc.scalar.copy(onesb[:, :], ones_f[:, :])
            # tri01[p, j] = 1 if j >= p else 0 (keep keys <= query), two
            # adjacent copies so one 3D-AP multiply masks both heads.
            tri01_f = cp.tile([128, 128], F32, tag="tri01_f")
            nc.gpsimd.affine_select(tri01_f[:, :], ones_f[:, :],
                                    pattern=[[1, 128]],
                                    compare_op=Alu.is_ge, fill=0.0,
                                    base=0, channel_multiplier=-1)
            tri2 = cp.tile([128, 256], BF16, tag="tri2")
            nc.scalar.copy(tri2[:, 0:128], tri01_f[:, :])
            nc.scalar.copy(tri2[:, 128:256], tri01_f[:, :])
            qb_s = cp.tile([128, 4], F32, tag="qb")
            nc.sync.dma_start(qb_s[:, :], qb_d[:, :])
            kb_s = cp.tile([128, 4], F32, tag="kb")
            nc.sync.dma_start(kb_s[:, :], kb_d[:, :])
            vb_s = cp.tile([1, OG], BF16, tag="vb")
            nc.sync.dma_start(vb_s[:, :], vb_d[:, :])
            kmask_s = cp.tile([128, SC], F32, tag="km")
            nc.sync.dma_start(kmask_s[:, :], kmask_d[:, :])

            def load_x_quarter(q):
                # one batched DMA for the whole quarter (the Sync queue
                # costs ~600ns per dma_start issue)
                qs = q * W
                xf = xp.tile([128, KC * W], BF16, tag="xTf", name=f"xT{q}")
                nc.sync.dma_start(
                    xf[:, :].rearrange("p (kc s) -> p kc s", kc=KC),
                    xT_d[:, qs:qs + W].rearrange("(kc p) s -> p kc s", p=128))
                return [xf[:, kc * W:(kc + 1) * W] for kc in range(KC)]

            # quarter-0 activations first so the first projection matmuls
            # can start while the bulk of the weights still stream in
            x0_t = load_x_quarter(0)

            # ---------------- weights (one batched DMA each) ----------------
            def load_wflat(pool, src, nkc, width, name):
                flat = pool.tile([128, nkc * width], BF16, tag=name)
                nc.sync.dma_start(
                    flat[:, :].rearrange("p (kc s) -> p kc s", kc=nkc),
                    src.rearrange("(kc p) s -> p kc s", p=128))
                return [flat[:, kc * width:(kc + 1) * width]
                        for kc in range(nkc)]

            wq_t = load_wflat(wqp, wqT_d, KC, OG, "wqf")
            wk_t = load_wflat(wkp, wkT_d, KC, OG, "wkf")
            wv_t = load_wflat(wvp, wvT_d, KC, OG, "wvf")
            wo_t = load_wflat(wop, woT_d, 4, HID, "wof")

            # ---------------- persistent activation tiles ----------------
            qT_t = [qTp.tile([128, SEQ], BF16, tag=f"qT{i}", name=f"qT{i}")
                    for i in range(NPAIR)]
            kT_t = [kTp.tile([128, SEQ], BF16, tag=f"kT{i}", name=f"kT{i}")
                    for i in range(NPAIR)]
            v_t = [vp.tile([128, HPG * 65], BF16, tag=f"v{i}", name=f"v{i}")
                   for i in range(SC)]
            for i in range(SC):
                vv = v_t[i].rearrange("p (h c) -> p h c", c=65)
                nc.gpsimd.memset(vv[:, :, 64:65], 1.0)
            attnT_t = [aTp.tile([128, SEQ], BF16, tag=f"aT{i}", name=f"aT{i}")
                       for i in range(NPAIR)]

            def phase1_quarter(q, xT_t=None):
                qs = q * W
                if xT_t is None:
                    xT_t = load_x_quarter(q)
                # quarter 0 runs before any attention window, so the at
                # accumulator pool (psB, 3 slots) is idle -- borrow it to
                # avoid serializing on the single-slot psC pool
                pool, ptag = (psB, "b512") if q == 0 else (psC, "c512")
                # Q/K projections: out partitions = proj dims, cols = seq
                for w_t, o_t, bias in ((wq_t, qT_t, qb_s), (wk_t, kT_t, kb_s)):
                    for oc in range(4):
                        pqk = pool.tile([128, W], F32, tag=ptag, name="pqk")
                        for kc in range(KC):
                            nc.tensor.matmul(
                                pqk[:, :],
                                w_t[kc][:, oc * 128:(oc + 1) * 128],
                                xT_t[kc][:, :],
                                start=(kc == 0), stop=(kc == KC - 1))
                        nc.vector.tensor_scalar_add(
                            o_t[oc][:, qs:qs + W], pqk[:, :],
                            bias[:, oc:oc + 1])
                # V projection: out partitions = seq chunk, cols = proj dims
                for sc in range(4):
                    scg = 4 * q + sc
                    pv = pool.tile([128, OG], F32, tag=ptag, name="pv")
                    for kc in range(KC):
                        nc.tensor.matmul(
                            pv[:, :],
                            xT_t[kc][:, sc * 128:(sc + 1) * 128],
                            wv_t[kc][:, :],
                            start=(kc == 0), stop=False)
                    nc.tensor.matmul(pv[:, :], onesb[0:1, :], vb_s[0:1, :],
                                     start=False, stop=True)
                    src = pv.rearrange("p (h c) -> p h c", c=64)
                    dst = v_t[scg].rearrange("p (h c) -> p h c", c=65)
                    nc.vector.tensor_copy(dst[:, :, 0:64], src[:, :, :])

            def phase2_window(w, psA):
                ws = w * W
                chunks = [(c, 0) for c in range(4 * w)]
                chunks += [(4 * w + i, 128 * i) for i in range(4)]
                last = len(chunks) - 1
                for pr in range(NPAIR):
                    he = 2 * pr       # even head (rows 0:64)
                    at_e = psB.tile([128, W], F32, tag="b512", name="at_e")
                    at_o = psB.tile([128, W], F32, tag="b512", name="at_o")
                    for idx, (c, off) in enumerate(chunks):
                        n = W - off
                        sp = psA.tile([128, 2 * W], F32, tag="sp", name="sp")
                        sp3 = sp.rearrange("p (g c) -> p g c", g=2)
                        nc.tensor.matmul(
                            sp[:, off:W],
                            kT_t[pr][0:64, c * 128:(c + 1) * 128],
                            qT_t[pr][0:64, ws + off:ws + W],
                            start=True, stop=True)
                        nc.tensor.matmul(
                            sp[:, W + off:2 * W],
                            kT_t[pr][64:128, c * 128:(c + 1) * 128],
                            qT_t[pr][64:128, ws + off:ws + W],
                            start=True, stop=True)
                        et = p2.tile([128, 2 * W], BF16, tag="E", bufs=8)
                        et3 = et.rearrange("p (g c) -> p g c", g=2)
                        nc.scalar.activation(et3[:, :, off:W],
                                             sp3[:, :, off:W], AF.Exp,
                                             bias=kmask_s[:, c:c + 1],
                                             scale=SCALE)
                        if off or c == 4 * w:  # diagonal chunk
                            nc.vector.tensor_mul(
                                et3[:, :, off:off + 128],
                                et3[:, :, off:off + 128],
                                tri2.rearrange("p (g c) -> p g c", g=2))
                        nc.tensor.matmul(
                            at_e[0:65, off:W],
                            v_t[c][:, he * 65:(he + 1) * 65],
                            et[:, off:W],
                            start=(idx == 0), stop=(idx == last))
                        nc.tensor.matmul(
                            at_o[0:65, off:W],
                            v_t[c][:, (he + 1) * 65:(he + 2) * 65],
                            et[:, W + off:2 * W],
                            start=(idx == 0), stop=(idx == last))
                    # softmax division, off the PE critical path:
                    # evict unnormalized att + denominator row quickly
                    # (frees the PSUM slot), reciprocal on idle GpSimd,
                    # broadcast across partitions via a K=1 matmul.
                    for h, at in ((he, at_e), (he + 1, at_o)):
                        attnU = p2.tile([64, W], BF16, tag="aU", bufs=4)
                        nc.vector.tensor_copy(attnU[:, :], at[0:64, :])
                        dnr = p2.tile([128, W], F32, tag="dnr", bufs=2)
                        nc.vector.tensor_copy(dnr[64:65, :], at[64:65, :])
                        # reciprocal: DVE divide costs 8 cyc per FREE elem,
                        # so reshape the row to [128, 4] via DMA first
                        dnT = p2.tile([128, 4], F32, tag="dnT", bufs=2)
                        nc.sync.dma_start(dnT[:, :], dnr[64:65, :])
                        dnTr = p2.tile([128, 4], BF16, tag="dnTr", bufs=2)
                        with nc.allow_low_precision("recip"):
                            nc.vector.reciprocal(dnTr[:, :], dnT[:, :])
                        rcp = p2.tile([128, W], BF16, tag="rcp", bufs=2)
                        nc.sync.dma_start(rcp[64:65, :], dnTr[:, :])
                        # broadcast the reciprocal row back into the (now
                        # dead) at tile -- reuses its PSUM bank, WAW-ordered
                        # behind the two evictions above
                        nc.tensor.matmul(at[0:64, :], onesb[64:65, 0:64],
                                         rcp[64:65, :], start=True, stop=True)
                        if h % 2 == 0:
                            nc.vector.tensor_mul(
                                attnT_t[pr][0:64, ws:ws + W],
                                attnU[:, :], at[0:64, :])
                        else:
                            tmp = p2.tile([64, W], BF16, tag="tm", bufs=3)
                            nc.vector.tensor_mul(tmp[:, :], attnU[:, :],
                                                 at[0:64, :])
                            nc.sync.dma_start(attnT_t[pr][64:128, ws:ws + W],
                                              tmp[:, :])

            def phase3_window(w, pool, tag):
                # output projection for the sq chunks of window w
                for sc in range(4 * w, 4 * w + 4):
                    ot = p3.tile([128, HID], BF16, tag="ou", bufs=3)
                    for n in range(2):
                        po = pool.tile([128, W], F32, tag=tag, name="po")
                        for kc in range(4):
                            nc.tensor.matmul(
                                po[:, :],
                                attnT_t[kc][:, sc * 128:(sc + 1) * 128],
                                wo_t[kc][:, n * W:(n + 1) * W],
                                start=(kc == 0), stop=(kc == 3))
                        nc.vector.tensor_copy(ot[:, n * W:(n + 1) * W],
                                              po[:, :])
                    nc.sync.dma_start(out_d[sc * 128:(sc + 1) * 128, :],
                                      ot[:, :])

            # interleave projection quarters, attention windows, and output
            # projection so the scheduler can fill PE idle time during
            # ScalarE-bound (softmax) stretches; the last window's output
            # projection runs after the scores pool closes, in a wider pool
            with tc.tile_pool(name="psA", bufs=2, space="PSUM") as psA:
                phase1_quarter(0, x0_t)
                phase2_window(0, psA)
                phase1_quarter(1)
                phase2_window(1, psA)
                phase1_quarter(2)
                phase2_window(2, psA)
                phase3_window(0, psC, "c512")
                phase1_quarter(3)
                phase2_window(3, psA)
                phase3_window(1, psC, "c512")
                phase3_window(2, psC, "c512")
            with tc.tile_pool(name="psD", bufs=4, space="PSUM") as psD:
                phase3_window(3, psD, "d512")

    nc.compile()
    return nc


def kernel(hidden_states, causal_mask, padding_mask,
           q_w, q_b, k_w, k_b, v_w, v_b, o_w, o_b):
    global _compiled
    from concourse.bass_utils import run_bass_kernel_spmd
    import ml_dtypes

    BF = ml_dtypes.bfloat16

    hidden_states = np.asarray(hidden_states, dtype=np.float32)
    padding_mask = np.asarray(padding_mask)
    q_w = np.asarray(q_w, dtype=np.float32)
    k_w = np.asarray(k_w, dtype=np.float32)
    v_w = np.asarray(v_w, dtype=np.float32)
    o_w = np.asarray(o_w, dtype=np.float32)
    q_b = np.asarray(q_b, dtype=np.float32)
    k_b = np.asarray(k_b, dtype=np.float32)
    v_b = np.asarray(v_b, dtype=np.float32)
    o_b = np.asarray(o_b, dtype=np.float32)

    if _compiled is None:
        _compiled = _build()
    nc = _compiled

    in_maps = []
    for b in range(BS):
        xT = np.ascontiguousarray(hidden_states[b].T).astype(BF)
        kmask = np.where(padding_mask[b], np.float32(-30000.0),
                         np.float32(0.0)).astype(np.float32)
        kmask2 = np.ascontiguousarray(kmask.reshape(SC, 128).T)
        for g in range(HG):
            r = slice(g * OG, (g + 1) * OG)
            in_maps.append({
                "xT": xT,
                "wqT": np.ascontiguousarray(q_w[r].T).astype(BF),
                "wkT": np.ascontiguousarray(k_w[r].T).astype(BF),
                "wvT": np.ascontiguousarray(v_w[r].T).astype(BF),
                "woT": np.ascontiguousarray(o_w[:, r].T).astype(BF),
                "qb": np.ascontiguousarray(q_b[r].reshape(4, 128).T),
                "kb": np.ascontiguousarray(k_b[r].reshape(4, 128).T),
                "vb": np.ascontiguousarray(v_b[r].reshape(1, OG)).astype(BF),
                "kmask": kmask2,
            })

    trace = os.environ.get("KERNEL_TRACE") == "1"
    res = run_bass_kernel_spmd(nc, in_maps, core_ids=list(range(NCORES)),
                               trace=trace)
    if trace and res.exec_time_ns is not None:
        print(f"HW exec time: {res.exec_time_ns} ns")
        if res.instructions_and_trace:
            print(f"trace: {res.instructions_and_trace[1]}")

    out = np.empty((BS, SEQ, HID), dtype=np.float32)
    for b in range(BS):
        out[b] = (res.results[2 * b]["out"].astype(np.float32)
                  + res.results[2 * b + 1]["out"].astype(np.float32)
                  + o_b[None, :])
    return out
